# revision 1
# baseline (speedup 1.0000x reference)
"""CovPool kernel for 8 TRN2 NeuronCores.

reference semantics (B=32, N=16384, D=64):
    cov_b = (X_b - mean_b)^T (X_b - mean_b) / (N-1) + lam*I        (64x64)
    out   = sort(concat_b triu(cov_b)) reshaped to (B, 2080)

Device strategy (data parallel over batch):
  - core c owns batches [4c, 4c+4): streams its 16 MB slab once,
    accumulating per batch   P = [X|1]^T [X|1]  (65x65 PSUM, 128 matmuls)
    which yields G = X^T X, s = col/row sums, then one K=1 matmul adds
    -s s^T / N into PSUM so PSUM = (N-1)*cov - (N-1)*lam*I.
  - DVE: cov = PSUM * 1/(N-1) + lam*I, gpsimd affine_select masks the
    strict lower triangle to +BIG (sort padding).
  - (v1) masked cov tiles are DMAed out; host extracts triu + sorts.
"""

import sys

sys.path.insert(0, "/opt/trn_rl_repo")

import numpy as np

from concourse import bacc, mybir
from concourse.tile import TileContext
from concourse.bass_utils import run_bass_kernel_spmd

B, N, D = 32, 16384, 64
NCORES = 8
BPC = B // NCORES  # batches per core
LAMBDA = 0.01
D_OUT = D * (D + 1) // 2  # 2080
BIG = 3.0e38  # lower-triangle fill (sorts above every real value)

ROWS_PER_TILE = 128
TILES_PER_BATCH = N // ROWS_PER_TILE  # 128
R_PER_PART = 8  # consecutive x-rows held per partition per stream buffer
ROWS_PER_DMA = 128 * R_PER_PART  # 1024 rows = 256 KB per dma_start
DMAS_PER_BATCH = N // ROWS_PER_DMA  # 16
NSTREAM = 6  # stream ring depth

f32 = mybir.dt.float32
bf16 = mybir.dt.bfloat16


def _emit_cov_body(tc, nc, x, out, stream, ones_col, lam_tile,
                   work_pool, psum_pool, variant="full"):
    """One full covariance pass: stream all batches, write masked cov.

    Stream buffer layout: (128 partitions, 512) where partition p holds x
    rows [base + p*8, base + p*8 + 8) -- a fully contiguous 256 KB DMA.
    Slice r (cols [64r, 64r+64)) holds rows {base + p*8 + r} over p, so
    the 8 per-slice Gram matmuls together contract over all 1024 rows.
    A ones-vector matmul per DMA accumulates per-(r,d) column sums into
    psum row 64; they're folded to s at batch end.
    """
    di = 0  # global DMA counter for ring indexing
    stream_f32, stream_bf = stream
    xf = x.rearrange("b n d -> b (n d)")  # flat per-batch view
    for b in range(BPC):
        psum = psum_pool.tile([D + 1, R_PER_PART * D], f32,
                              tag=f"acc{b % 4}")
        for t in range(DMAS_PER_BATCH):
            buf = stream_f32[di % NSTREAM]
            bbuf = stream_bf[di % NSTREAM]
            eng = nc.sync if di % 2 == 0 else nc.scalar
            di += 1
            if variant != "mm_only":
                eng.dma_start(
                    buf[:],
                    xf[b, t * ROWS_PER_DMA * D:(t + 1) * ROWS_PER_DMA * D]
                    .rearrange("(p f) -> p f", p=128),
                )
            if variant == "dma_only":
                continue
            # fp32 -> bf16 cast on the (otherwise idle) scalar engine
            nc.scalar.copy(bbuf[:], buf[:])
            for r in range(R_PER_PART):
                nc.tensor.matmul(
                    psum[0:D, 0:D], bbuf[:, r * D:(r + 1) * D],
                    bbuf[:, r * D:(r + 1) * D],
                    start=(t == 0 and r == 0), stop=False,
                )
            nc.tensor.matmul(
                psum[D:D + 1, :], ones_col[:], bbuf[:],
                start=(t == 0), stop=(t == DMAS_PER_BATCH - 1),
            )
        if variant == "dma_only":
            continue
        # fold the (r, d) column sums to s (1, 64): sum over r
        s_sb = work_pool.tile([1, D], f32, tag="s_sb")
        s_neg = work_pool.tile([1, D], f32, tag="s_neg")
        s_view = psum[D:D + 1, :].rearrange("p (r d) -> p d r", d=D)
        nc.vector.tensor_reduce(
            out=s_sb[:], in_=s_view, axis=mybir.AxisListType.X,
            op=mybir.AluOpType.add,
        )
        nc.scalar.mul(s_neg[:], s_sb[:], -1.0 / N)
        nc.tensor.matmul(
            psum[0:D, 0:D], s_sb[:], s_neg[:],
            start=False, stop=True,
        )
        # cov = PSUM/(N-1) + lam*I
        cov_sb = work_pool.tile([D, D], f32, tag="cov")
        nc.vector.scalar_tensor_tensor(
            out=cov_sb[:], in0=psum[0:D, 0:D], scalar=1.0 / (N - 1),
            in1=lam_tile[:], op0=mybir.AluOpType.mult,
            op1=mybir.AluOpType.add,
        )
        # mask strict lower triangle (j < i) to BIG
        masked = work_pool.tile([D, D], f32, tag="masked")
        nc.gpsimd.affine_select(
            out=masked[:], in_=cov_sb[:], pattern=[[1, D]],
            compare_op=mybir.AluOpType.is_ge, fill=BIG,
            base=0, channel_multiplier=-1,
        )
        nc.sync.dma_start(out[b], masked[:])


def build_cov_kernel(bench_reps=None, variant="full"):
    nc = bacc.Bacc("TRN2", target_bir_lowering=False, debug=False,
                   num_devices=NCORES)
    x = nc.dram_tensor("x", [BPC, N, D], f32, kind="ExternalInput")
    out = nc.dram_tensor("out", [BPC, D, D], f32, kind="ExternalOutput")

    with TileContext(nc) as tc:
        with (
            tc.tile_pool(name="stream", bufs=1) as stream_pool,
            tc.tile_pool(name="const", bufs=1) as const_pool,
            tc.tile_pool(name="work", bufs=2) as work_pool,
            tc.tile_pool(name="psum", bufs=1, space="PSUM") as psum_pool,
        ):
            # constants
            lam_tile = const_pool.tile([D, D], f32, tag="lam")
            nc.vector.memset(lam_tile[:], LAMBDA)
            # keep lam only on the diagonal: iota = j - i, keep where ==0
            nc.gpsimd.affine_select(
                out=lam_tile[:], in_=lam_tile[:], pattern=[[1, D]],
                compare_op=mybir.AluOpType.is_equal, fill=0.0,
                base=0, channel_multiplier=-1,
            )

            ones_col = const_pool.tile([128, 1], bf16, tag="ones")
            nc.vector.memset(ones_col[:], 1.0)

            # stream ring: (128, 512) tiles, 8 consecutive rows/partition
            stream_f32 = [
                stream_pool.tile([128, R_PER_PART * D], f32,
                                 tag=f"stream{i}", name=f"stream{i}")
                for i in range(NSTREAM)
            ]
            stream_bf = [
                stream_pool.tile([128, R_PER_PART * D], bf16,
                                 tag=f"streambf{i}", name=f"streambf{i}")
                for i in range(NSTREAM)
            ]
            stream = (stream_f32, stream_bf)

            if variant == "mm_only":
                for t in stream_f32:
                    nc.vector.memset(t[:], 0.5)

            def body():
                _emit_cov_body(tc, nc, x, out, stream, ones_col, lam_tile,
                               work_pool, psum_pool, variant=variant)
                if variant == "dma_only":
                    # consume the stream buffers so Tile sees a reader,
                    # and produce the declared output
                    scrap = work_pool.tile([128, 1], f32, tag="scrap")
                    for t in stream_f32:
                        nc.vector.tensor_reduce(
                            out=scrap[:], in_=t[:],
                            axis=mybir.AxisListType.X,
                            op=mybir.AluOpType.max,
                        )
                    for b in range(BPC):
                        nc.sync.dma_start(out[b], stream_f32[0][0:D, 0:D])

            if bench_reps is None:
                body()
            else:
                with tc.For_i(0, bench_reps, 1):
                    body()

    nc.compile()
    return nc


_NC_CACHE = {}


def _get_kernel():
    if "nc" not in _NC_CACHE:
        _NC_CACHE["nc"] = build_cov_kernel()
    return _NC_CACHE["nc"]


def run_device(x_full: np.ndarray):
    """Run the bass kernel on 8 cores; returns per-core masked cov tiles,
    list of (BPC, D, D)."""
    nc = _get_kernel()
    in_maps = [
        {"x": np.ascontiguousarray(x_full[c * BPC:(c + 1) * BPC])}
        for c in range(NCORES)
    ]
    res = run_bass_kernel_spmd(nc, in_maps, core_ids=list(range(NCORES)))
    return [res.results[c]["out"] for c in range(NCORES)]


def kernel(x: np.ndarray) -> np.ndarray:
    x = np.asarray(x, dtype=np.float32)
    covs = run_device(x)  # 8 x (BPC, D, D), lower tri = BIG
    all_cov = np.concatenate(covs, axis=0)  # (B, D, D)
    iu, ju = np.triu_indices(D)
    tri = all_cov[:, iu, ju]  # (B, D_OUT)
    return np.sort(tri.reshape(-1)).reshape(B, D_OUT).astype(np.float32)


if __name__ == "__main__":
    rng = np.random.default_rng(0)
    xt = rng.standard_normal((B, N, D), dtype=np.float32)
    out = kernel(xt)
    print("kernel out shape:", out.shape, out.dtype)



# revision 34
# speedup vs baseline: 59.4397x; 59.4397x over previous
"""CovPool kernel for 8 TRN2 NeuronCores.

reference semantics (B=32, N=16384, D=64):
    cov_b = (X_b - mean_b)^T (X_b - mean_b) / (N-1) + lam*I        (64x64)
    out   = sort(concat_b triu(cov_b)) reshaped to (B, 2080)

Device strategy (data parallel over batch):
  - core c owns batches [4c, 4c+4): streams its 16 MB slab once in
    1 MiB DMAs spread across SP/ACT/Pool (the first two buffers arrive
    as parallel quarter/half DMAs to shorten pipeline fill). Each f32
    buffer [128, 2048] is cast to bf16 slices of 65 cols (64 x-cols +
    a preset ones col), mostly on DVE with an ACT share.
  - 32 Gram matmuls per buffer accumulate G' = [X|1]^T[X|1] into a
    65x65 PSUM tile (128 matmuls per batch); PSUM is DMAed out raw.
  - Host finishes: cov = (G - s s^T / N)/(N-1) + lam*I with
    G = G'[0:64,0:64], s = G'[64,0:64]; then triu + global sort.
"""

import sys

sys.path.insert(0, "/opt/trn_rl_repo")

import numpy as np

from concourse import bacc, mybir
from concourse.tile import TileContext
from concourse.bass_utils import run_bass_kernel_spmd

B, N, D = 32, 16384, 64
NCORES = 8
BPC = B // NCORES  # batches per core
LAMBDA = 0.01
D_OUT = D * (D + 1) // 2  # 2080

R_PER_PART = 32  # consecutive x-rows held per partition per stream buffer
ROWS_PER_DMA = 128 * R_PER_PART  # 4096 rows = 1 MiB per dma_start
DMAS_PER_BATCH = N // ROWS_PER_DMA  # 4
NDMA = BPC * DMAS_PER_BATCH  # 16
NSTREAM = 8  # stream ring depth
DW = D + 1  # 65: x cols + ones col per slice

# cast plan, keyed by DMA index mod 4: (engine, lo, hi) slice ranges
# covering [0, R_PER_PART). DVE is the main caster (2 pieces for finer
# matmul deps); ACT takes a tail share on every 4th buffer.
CAST_PLAN = {
    0: (("vector", 0, 8), ("vector", 8, 16), ("vector", 16, 24),
        ("vector", 24, 32)),
}
# pipeline-fill accelerators: buffer 0 arrives as 4 parallel quarter
# DMAs, buffer 1 as 2 halves; casts align to the sub-DMA boundaries
DMA_SPLITS = {
    0: (("sync", 0, 8), ("gpsimd", 8, 16), ("scalar", 16, 24),
        ("sync", 24, 32)),
    1: (("gpsimd", 0, 16), ("scalar", 16, 32)),
}
SPLIT_CAST_PLAN = {
    0: (("vector", 0, 8), ("vector", 8, 16), ("vector", 16, 24),
        ("vector", 24, 32)),
    1: (("vector", 0, 8), ("vector", 8, 16), ("vector", 16, 24),
        ("vector", 24, 32)),
}
# issuing engine per stream DMA (entries 0/1 unused - see DMA_SPLITS);
# DMA-capable engines are SP (sync), ACT (scalar), Pool (gpsimd) only
DMA_ASSIGN = ("sync", "sync", "gpsimd", "scalar",
              "sync", "gpsimd", "scalar", "sync",
              "gpsimd", "sync", "scalar", "gpsimd",
              "sync", "gpsimd", "scalar", "sync")

f32 = mybir.dt.float32
bf16 = mybir.dt.bfloat16


def _emit_cov_body(tc, nc, x, out, stream, work_pool, psum_pool,
                   variant="full"):
    """One full covariance pass: stream all batches, write raw G' tiles."""
    di = 0  # global DMA counter for ring indexing
    stream_f32, stream_bf = stream
    xf = x.rearrange("b n d -> b (n d)")  # flat per-batch view
    for b in range(BPC):
        # col-tiled accumulation: even slices -> psum[0:64] (col grp 0-1),
        # odd slices -> psum[64:128] (col grp 2-3); halves run
        # concurrently in the PE array and are summed at batch end
        psum = psum_pool.tile([128, DW], f32, tag=f"acc{b}")
        for t in range(DMAS_PER_BATCH):
            buf = stream_f32[di % NSTREAM]
            bbuf = stream_bf[di % NSTREAM]
            dma_eng = getattr(nc, DMA_ASSIGN[di % len(DMA_ASSIGN)])
            plan = CAST_PLAN[di % len(CAST_PLAN)]
            if di in DMA_SPLITS:
                plan = SPLIT_CAST_PLAN[di]
            src = (xf[b, t * ROWS_PER_DMA * D:(t + 1) * ROWS_PER_DMA * D]
                   .rearrange("(p f) -> p f", p=128))
            if variant != "mm_only":
                if di in DMA_SPLITS:
                    for eng_name, s0, s1 in DMA_SPLITS[di]:
                        getattr(nc, eng_name).dma_start(
                            buf[:, s0 * D:s1 * D], src[:, s0 * D:s1 * D])
                else:
                    dma_eng.dma_start(buf[:], src)
            di += 1
            if variant == "dma_only":
                continue
            if variant not in ("mm_only", "no_cast"):
                # fp32 -> bf16 cast; dst is the 64-col part of each
                # 65-wide slice (col 64 holds preset 1.0)
                fview = buf[:].rearrange("p (r c) -> p r c", c=D)
                bview = bbuf[:].rearrange("p (r c) -> p r c", c=DW)
                for eng_name, c0, c1 in plan:
                    eng = getattr(nc, eng_name)
                    if eng_name == "scalar":
                        eng.copy(bview[:, c0:c1, 0:D], fview[:, c0:c1, :])
                    else:
                        eng.tensor_copy(bview[:, c0:c1, 0:D],
                                        fview[:, c0:c1, :])
            mm_step = 2 if variant == "half_mm" else 1
            rs = list(range(0, R_PER_PART, mm_step))
            for i, r in enumerate(rs):
                half = i % 2
                po = D * half
                nc.tensor.matmul(
                    psum[po:po + D, 0:DW],
                    bbuf[:, r * DW:r * DW + D],
                    bbuf[:, r * DW:(r + 1) * DW],
                    start=(t == 0 and i < 2),
                    stop=(t == DMAS_PER_BATCH - 1 and i >= len(rs) - 2),
                    tile_position=(0, po),
                )
        if variant == "dma_only":
            continue
        # merge the two col-tile halves and ship raw G' [64, 65] to HBM;
        # host finishes (G = [:, 0:64], s = [:, 64])
        g_sb = work_pool.tile([D, DW], f32, tag=f"g{b % 2}")
        nc.vector.tensor_copy(g_sb[:], psum[0:D, :])
        nc.vector.tensor_tensor(
            out=g_sb[:], in0=g_sb[:], in1=psum[D:2 * D, :],
            op=mybir.AluOpType.add,
        )
        out_eng = nc.sync if b % 2 == 0 else nc.scalar
        out_eng.dma_start(out[b], g_sb[:])


def build_cov_kernel(bench_reps=None, variant="full"):
    nc = bacc.Bacc("TRN2", target_bir_lowering=False, debug=False,
                   num_devices=NCORES)
    x = nc.dram_tensor("x", [BPC, N, D], f32, kind="ExternalInput")
    out = nc.dram_tensor("out", [BPC, D, DW], f32, kind="ExternalOutput")

    with TileContext(nc) as tc:
        with (
            tc.tile_pool(name="stream", bufs=1) as stream_pool,
            tc.tile_pool(name="work", bufs=2) as work_pool,
            tc.tile_pool(name="psum", bufs=1, space="PSUM") as psum_pool,
        ):
            # stream ring: f32 [128, R*64] + bf16 [128, R*65] tiles
            stream_f32 = [
                stream_pool.tile([128, R_PER_PART * D], f32,
                                 tag=f"stream{i}", name=f"stream{i}")
                for i in range(NSTREAM)
            ]
            stream_bf = [
                stream_pool.tile([128, R_PER_PART * DW], bf16,
                                 tag=f"streambf{i}", name=f"streambf{i}")
                for i in range(NSTREAM)
            ]
            # preset the ones column (col 64 of every 65-wide slice)
            for t in stream_bf:
                ones_view = t[:].rearrange("p (r c) -> p r c", c=DW)
                nc.gpsimd.memset(ones_view[:, :, D:DW], 1.0)
            stream = (stream_f32, stream_bf)

            if variant in ("mm_only", "no_cast"):
                for t in stream_bf:
                    nc.vector.memset(t[:], 0.5)

            def body():
                _emit_cov_body(tc, nc, x, out, stream, work_pool,
                               psum_pool, variant=variant)
                if variant == "no_cast":
                    scrap2 = work_pool.tile([128, 1], f32, tag="scrap2")
                    for t in stream_f32:
                        nc.vector.tensor_reduce(
                            out=scrap2[:], in_=t[:, 0:8],
                            axis=mybir.AxisListType.X,
                            op=mybir.AluOpType.max,
                        )
                if variant == "dma_only":
                    # consume the stream buffers so Tile sees a reader,
                    # and produce the declared output
                    scrap = work_pool.tile([128, 1], f32, tag="scrap")
                    for t in stream_f32:
                        nc.vector.tensor_reduce(
                            out=scrap[:], in_=t[:, 0:8],
                            axis=mybir.AxisListType.X,
                            op=mybir.AluOpType.max,
                        )
                    for b in range(BPC):
                        nc.sync.dma_start(out[b], stream_f32[0][0:D, 0:DW])

            if bench_reps is None:
                body()
            else:
                with tc.For_i(0, bench_reps, 1):
                    body()

    nc.compile()
    return nc


_NC_CACHE = {}


def _get_kernel():
    if "nc" not in _NC_CACHE:
        _NC_CACHE["nc"] = build_cov_kernel()
    return _NC_CACHE["nc"]


def run_device(x_full: np.ndarray):
    """Run the bass kernel on 8 cores; returns per-core raw G' tiles,
    list of (BPC, D, DW)."""
    nc = _get_kernel()
    in_maps = [
        {"x": np.ascontiguousarray(x_full[c * BPC:(c + 1) * BPC])}
        for c in range(NCORES)
    ]
    res = run_bass_kernel_spmd(nc, in_maps, core_ids=list(range(NCORES)))
    return [res.results[c]["out"] for c in range(NCORES)]


def postprocess(raw: np.ndarray) -> np.ndarray:
    """(B, 64, 65) raw G' -> (B, D_OUT) sorted triu of cov."""
    raw = raw.astype(np.float64)
    G = raw[:, :, 0:D]
    s = raw[:, :, D]
    cov = (G - s[:, :, None] * s[:, None, :] / N) / (N - 1)
    cov += LAMBDA * np.eye(D)
    iu, ju = np.triu_indices(D)
    tri = cov[:, iu, ju]  # (B, D_OUT)
    return np.sort(tri.reshape(-1)).reshape(B, D_OUT).astype(np.float32)


def kernel(x: np.ndarray) -> np.ndarray:
    x = np.asarray(x, dtype=np.float32)
    raws = run_device(x)  # 8 x (BPC, DW, DW)
    return postprocess(np.concatenate(raws, axis=0))


if __name__ == "__main__":
    rng = np.random.default_rng(0)
    xt = rng.standard_normal((B, N, D), dtype=np.float32)
    out = kernel(xt)
    print("kernel out shape:", out.shape, out.dtype)
